# revision 1
# baseline (speedup 1.0000x reference)
"""Cubic B-spline interpolation kernel for Trainium2 (Bass/Tile), 8 cores.

Reference computation: for each of 2M points, evaluate a cardinal cubic
B-spline on a 132^3 control grid (4x4x4 stencil per point).

Strategy (data-parallel over points, grid replicated per core):
  - Host: shard points into 8 contiguous slices of 250,000, pad each to
    250,880 = 128 partitions x 1960 slots.
  - Device, per 16-slot chunk: compute floor/frac/weights on DVE, build the
    stencil-corner flat index, then gather per (point, i-plane) one
    contiguous 400-float run G.flat[corner + i*132^2 : +400] via indirect
    DMA (one descriptor per partition, the verified n_idx=1 form). The 400
    run covers the whole 4x4 (y,z) patch at static offsets j*132+k, so the
    tensor-product contraction is pure static-AP DVE work.
  - Output [128 x 1960] per core; host unshards/unpads.
"""

import numpy as np

GRID = 132
G2 = GRID * GRID  # 17424
NCELLS = GRID ** 3
P = 128
SLOTS = 1960
NPTS_CORE = 250_000
NPAD_CORE = P * SLOTS  # 250880
NC = 10  # slots per chunk
NCHUNK = SLOTS // NC  # 140
RUN = 1600  # run in T4 covering the full 4x4x4 stencil: (3*132+3)*4 + 4
T4SIZE = 129 * G2 * 4  # x-interleaved table [129, 132, 132, 4]

_CACHE = {}


def _build_program(nchunks=NCHUNK):
    from contextlib import ExitStack

    import concourse.bass as bass
    import concourse.tile as tile
    from concourse import bacc, mybir

    nc = bacc.Bacc("TRN2", num_devices=8, debug=False, target_bir_lowering=False)
    pts_d = nc.dram_tensor("pts", [NPAD_CORE, 3], mybir.dt.float32, kind="ExternalInput")
    g_d = nc.dram_tensor("grid", [T4SIZE, 1], mybir.dt.float32, kind="ExternalInput")
    out_d = nc.dram_tensor("out", [P, SLOTS], mybir.dt.float32, kind="ExternalOutput")

    f32 = mybir.dt.float32
    AL = mybir.AluOpType

    def sap(ap, pattern, off=0):
        v = ap.copy()
        v.ap = type(v.ap)(pattern)
        v.offset = v.offset + off
        return v

    with tile.TileContext(nc) as tc:
        with ExitStack() as ctx:
            cpool = ctx.enter_context(tc.tile_pool(name="cpool", bufs=1))
            pool = ctx.enter_context(tc.tile_pool(name="pool", bufs=2))
            xpool = ctx.enter_context(tc.tile_pool(name="xpool", bufs=2))

            for c in range(nchunks):
                pts_t = pool.tile([P, NC, 3], f32, tag="pts")
                # src: partition p -> rows p*SLOTS + c*NC .. +NC
                src = sap(pts_d[:], [[SLOTS * 3, P], [3, NC], [1, 3]], c * NC * 3)
                nc.sync.dma_start(pts_t[:], src)

                t_t = pool.tile([P, NC, 3], f32, tag="t")
                nc.vector.tensor_scalar_add(t_t[:], pts_t[:], 1.0)
                r_t = pool.tile([P, NC, 3], f32, tag="r")
                nc.vector.tensor_scalar(
                    r_t[:], t_t[:], 8388608.0, 8388608.0, op0=AL.add, op1=AL.subtract
                )
                gt_t = pool.tile([P, NC, 3], f32, tag="gt")
                nc.vector.tensor_tensor(gt_t[:], r_t[:], t_t[:], op=AL.is_gt)
                tif_t = pool.tile([P, NC, 3], f32, tag="tif")
                nc.vector.tensor_sub(tif_t[:], r_t[:], gt_t[:])
                frac_t = pool.tile([P, NC, 3], f32, tag="frac")
                nc.vector.tensor_sub(frac_t[:], t_t[:], tif_t[:])

                # weights -> W [P, NC, 3(dim), 4(tap)]
                W = pool.tile([P, NC, 3, 4], f32, tag="W")
                omx = pool.tile([P, NC, 3], f32, tag="omx")
                nc.vector.tensor_scalar(
                    omx[:], frac_t[:], -1.0, -1.0, op0=AL.mult, op1=AL.subtract
                )  # omx = -x - (-1) ... careful: (x*-1) - (-1) = 1 - x
                x2 = pool.tile([P, NC, 3], f32, tag="x2")
                nc.vector.tensor_mul(x2[:], frac_t[:], frac_t[:])
                x3 = pool.tile([P, NC, 3], f32, tag="x3")
                nc.vector.tensor_mul(x3[:], x2[:], frac_t[:])
                o2 = pool.tile([P, NC, 3], f32, tag="o2")
                nc.vector.tensor_mul(o2[:], omx[:], omx[:])
                o3 = pool.tile([P, NC, 3], f32, tag="o3")
                nc.vector.tensor_mul(o3[:], o2[:], omx[:])

                SIX = 1.0 / 6.0
                # c0 = o3/6 -> W[..., 0]
                nc.vector.tensor_scalar_mul(W[:, :, :, 0], o3[:], SIX)
                # c3 = x3/6 -> W[..., 3]
                nc.vector.tensor_scalar_mul(W[:, :, :, 3], x3[:], SIX)
                # c1 = 0.5*x3 - x2 + 2/3 -> W[..., 1]
                c1a = pool.tile([P, NC, 3], f32, tag="c1a")
                nc.vector.scalar_tensor_tensor(
                    c1a[:], x3[:], 0.5, x2[:], op0=AL.mult, op1=AL.subtract
                )
                nc.vector.tensor_scalar_add(W[:, :, :, 1], c1a[:], 2.0 / 3.0)
                # c2 = 0.5*o3 - o2 + 2/3 -> W[..., 2]
                c2a = pool.tile([P, NC, 3], f32, tag="c2a")
                nc.vector.scalar_tensor_tensor(
                    c2a[:], o3[:], 0.5, o2[:], op0=AL.mult, op1=AL.subtract
                )
                nc.vector.tensor_scalar_add(W[:, :, :, 2], c2a[:], 2.0 / 3.0)

                # corner index (f32 exact): ((bx*132)+by)*132+bz - 17557
                bx = tif_t[:, :, 0]
                by = tif_t[:, :, 1]
                bz = tif_t[:, :, 2]
                f1 = pool.tile([P, NC], f32, tag="f1")
                nc.vector.scalar_tensor_tensor(
                    f1[:], by, float(GRID), bz, op0=AL.mult, op1=AL.add
                )
                f2 = pool.tile([P, NC], f32, tag="f2")
                nc.vector.scalar_tensor_tensor(
                    f2[:], bx, float(G2), f1[:], op0=AL.mult, op1=AL.add
                )
                basef = pool.tile([P, NC], f32, tag="basef")
                nc.vector.tensor_scalar(
                    basef[:], f2[:], float(-(G2 + GRID + 1)), 4.0,
                    op0=AL.add, op1=AL.mult,
                )
                idxi = pool.tile([P, NC], mybir.dt.int32, tag="idxi")
                nc.vector.tensor_copy(idxi[:], basef[:])

                # gather: one desc/partition per point of RUN floats from T4
                X = xpool.tile([P, NC, RUN], f32, tag="X")
                for n in range(NC):
                    nc.gpsimd.indirect_dma_start(
                        out=X[:, n, :],
                        out_offset=None,
                        in_=g_d[:],
                        in_offset=bass.IndirectOffsetOnAxis(
                            ap=idxi[:, n : n + 1], axis=0
                        ),
                    )

                # contraction: patch(j,k) at offsets j*132+k within each run
                m1 = xpool.tile([P, NC, 4, 4, 4], f32, tag="m1")
                for i in range(4):
                    Xp_i = sap(
                        X[:],
                        [[NC * RUN, P], [RUN, NC], [GRID * 4, 4], [4, 4]],
                        i,
                    )
                    wz = sap(
                        W[:], [[NC * 12, P], [12, NC], [0, 4], [1, 4]], 2 * 4
                    )
                    nc.vector.tensor_tensor(m1[:, :, i, :, :], Xp_i, wz, op=AL.mult)
                A = pool.tile([P, NC, 4, 4], f32, tag="A")
                nc.vector.tensor_reduce(
                    A[:].rearrange("p n i j -> p (n i j)"),
                    m1[:].rearrange("p n i j k -> p (n i j) k"),
                    axis=mybir.AxisListType.X,
                    op=AL.add,
                )
                wy = sap(W[:], [[NC * 12, P], [12, NC], [0, 4], [1, 4]], 1 * 4)
                m2 = pool.tile([P, NC, 4, 4], f32, tag="m2")
                nc.vector.tensor_tensor(m2[:], A[:], wy, op=AL.mult)
                B = pool.tile([P, NC, 4], f32, tag="B")
                nc.vector.tensor_reduce(
                    B[:].rearrange("p n i -> p (n i)"),
                    m2[:].rearrange("p n i j -> p (n i) j"),
                    axis=mybir.AxisListType.X,
                    op=AL.add,
                )
                wx = sap(W[:], [[NC * 12, P], [12, NC], [1, 4]])
                m3 = pool.tile([P, NC, 4], f32, tag="m3")
                nc.vector.tensor_tensor(m3[:], B[:], wx, op=AL.mult)
                v = pool.tile([P, NC], f32, tag="v")
                nc.vector.tensor_reduce(
                    v[:],
                    m3[:],
                    axis=mybir.AxisListType.X,
                    op=AL.add,
                )
                dst = sap(out_d[:], [[SLOTS, P], [1, NC]], c * NC)
                nc.sync.dma_start(dst, v[:])

    nc.compile()
    return nc


def kernel(pts: np.ndarray, control_pts: np.ndarray) -> np.ndarray:
    from concourse.bass_utils import run_bass_kernel_spmd

    if "nc" not in _CACHE:
        _CACHE["nc"] = _build_program()
    nc = _CACHE["nc"]

    pts = np.ascontiguousarray(pts, dtype=np.float32)
    g3 = np.ascontiguousarray(control_pts, dtype=np.float32).reshape(GRID, GRID, GRID)
    # x-interleaved table: T4[xs, y, z, c] = G[xs+c, y, z] -> full stencil in
    # one contiguous 1600-float run at 4*(x0*G2 + y0*GRID + z0)
    t4 = np.stack([g3[c : 129 + c] for c in range(4)], axis=-1)
    t4 = np.ascontiguousarray(t4, np.float32).reshape(T4SIZE, 1)

    in_maps = []
    for k in range(8):
        sl = pts[k * NPTS_CORE : (k + 1) * NPTS_CORE]
        pad = np.zeros((NPAD_CORE, 3), np.float32)
        pad[: sl.shape[0]] = sl
        in_maps.append({"pts": pad, "grid": t4})

    res = run_bass_kernel_spmd(nc, in_maps, core_ids=list(range(8)))
    outs = []
    for k in range(8):
        o = res.results[k]["out"].reshape(NPAD_CORE)
        outs.append(o[:NPTS_CORE])
    return np.concatenate(outs).reshape(-1, 1)



# revision 2
# speedup vs baseline: 669.4729x; 669.4729x over previous
"""Cubic B-spline interpolation kernel v2 for Trainium2 (Bass/Tile), 8 cores.

Per core (250k points, grid replicated):
  1. Device builds a bf16 "T64" table in DRAM scratch: for every stencil base
     cell (bx,by,bz) in [0,128)^3, the 64 stencil values
     T64[cell*64 + cx*16 + cy*4 + cz] = G[bx+cx, by+cy, bz+cz].
     Built partition-parallel over bx from 4 y-shifted bf16 grid copies in
     SBUF via DVE interleave copies + bulk DMA out (268MB, ~1.5ms).
  2. Main loop over 35 chunks x 56 slots: compute floor/frac/weights on DVE,
     cell index as f32 (exact: cell*64 < 2^27), convert to int32; per slot one
     indirect DMA gathers the 128B stencil row per partition (verified
     n_idx=1 form); tensor-product contraction (z,y,x staged mul+reduce).
  3. Output [128, 1960] f32 per core; host unshards/unpads.

Timing support: build_program(reps=R) wraps the whole computation in a
hardware For_i loop; test harness measures (wall(R2)-wall(R1))/(R2-R1).
"""

import numpy as np

GRID = 132
GP = GRID * GRID  # 17424 elements per x-plane
P = 128
SLOTS = 1960
NPTS_CORE = 250_000
NPAD_CORE = P * SLOTS  # 250880
NC = 56  # slots per chunk
NCHUNK = SLOTS // NC  # 35
CELLS = 128 * 128 * 128
TSIZE = CELLS * 64  # bf16 elements: 134,217,728 (= 2^27, all idx exact in f32)
BY = 2  # by-block size for table build
XBUFS = 2  # gather-chunk pipeline depth

_CACHE = {}


def build_program(reps=1, nchunks=NCHUNK):
    from contextlib import ExitStack

    import concourse.bass as bass
    import concourse.tile as tile
    from concourse import bacc, mybir

    nc = bacc.Bacc("TRN2", num_devices=8, debug=False, target_bir_lowering=False)
    f32 = mybir.dt.float32
    bf16 = mybir.dt.bfloat16
    i32 = mybir.dt.int32
    AL = mybir.AluOpType

    pts_d = nc.dram_tensor("pts", [NPAD_CORE, 3], f32, kind="ExternalInput")
    g_d = nc.dram_tensor("gbf", [GRID, GP], bf16, kind="ExternalInput")
    out_d = nc.dram_tensor("out", [P, SLOTS], f32, kind="ExternalOutput")

    def sap(ap, pattern, off=0):
        v = ap.copy()
        v.ap = type(v.ap)(pattern)
        v.offset = v.offset + off
        return v

    with tile.TileContext(nc) as tc:
        with ExitStack() as octx:
            # persistent DRAM scratch for the table
            dpool = octx.enter_context(
                tc.tile_pool(name="dpool", bufs=1, space="DRAM")
            )
            t64 = dpool.tile([TSIZE, 1], bf16, tag="t64")

            def body(_=None):
                with ExitStack() as ctx:
                    # ---- phase 1: build T64 ----
                    gpool = ctx.enter_context(tc.tile_pool(name="gpool", bufs=1))
                    bpool = ctx.enter_context(tc.tile_pool(name="bpool", bufs=2))
                    Gx = gpool.tile([P, 4, GP], bf16, tag="Gx")
                    for cx in range(4):
                        # partition p <- plane p+cx
                        src = sap(g_d[:], [[GP, P], [1, GP]], cx * GP)
                        nc.sync.dma_start(Gx[:, cx, :], src)
                    for blk in range(128 // BY):
                        by0 = blk * BY
                        Tb = bpool.tile([P, BY, 128, 4, 4, 4], bf16, tag="Tb")
                        for cx in range(4):
                            for cy in range(4):
                                # dst [BY, bz=128, cz=4] at (cx, cy)
                                dst = sap(
                                    Tb[:],
                                    [[BY * 8192, P], [8192, BY], [64, 128], [1, 4]],
                                    cx * 16 + cy * 4,
                                )
                                srcv = sap(
                                    Gx[:],
                                    [[4 * GP, P], [GRID, BY], [1, 128], [1, 4]],
                                    cx * GP + (by0 + cy) * GRID,
                                )
                                nc.vector.tensor_copy(dst, srcv)
                        ddst = sap(
                            t64[:],
                            [[1048576, P], [1, BY * 8192]],
                            by0 * 8192,
                        )
                        nc.sync.dma_start(ddst, Tb[:])

                with ExitStack() as ctx:
                    # ---- phase 2: main point loop ----
                    pool = ctx.enter_context(tc.tile_pool(name="pool", bufs=2))
                    xpool = ctx.enter_context(
                        tc.tile_pool(name="xpool", bufs=XBUFS)
                    )
                    for c in range(nchunks):
                        pts_t = pool.tile([P, NC, 3], f32, tag="pts")
                        src = sap(
                            pts_d[:],
                            [[SLOTS * 3, P], [3, NC], [1, 3]],
                            c * NC * 3,
                        )
                        nc.sync.dma_start(pts_t[:], src)

                        t_t = pool.tile([P, NC, 3], f32, tag="t")
                        nc.vector.tensor_scalar_add(t_t[:], pts_t[:], 1.0)
                        r_t = pool.tile([P, NC, 3], f32, tag="r")
                        nc.vector.tensor_scalar(
                            r_t[:], t_t[:], 8388608.0, 8388608.0,
                            op0=AL.add, op1=AL.subtract,
                        )
                        gt_t = pool.tile([P, NC, 3], f32, tag="gt")
                        nc.vector.tensor_tensor(gt_t[:], r_t[:], t_t[:], op=AL.is_gt)
                        tif_t = pool.tile([P, NC, 3], f32, tag="tif")
                        nc.vector.tensor_sub(tif_t[:], r_t[:], gt_t[:])
                        frac_t = pool.tile([P, NC, 3], f32, tag="frac")
                        nc.vector.tensor_sub(frac_t[:], t_t[:], tif_t[:])

                        # weights W [P, NC, 3, 4] f32
                        W = pool.tile([P, NC, 3, 4], f32, tag="W")
                        omx = pool.tile([P, NC, 3], f32, tag="omx")
                        nc.vector.tensor_scalar(
                            omx[:], frac_t[:], -1.0, -1.0,
                            op0=AL.mult, op1=AL.subtract,
                        )
                        x2 = pool.tile([P, NC, 3], f32, tag="x2")
                        nc.vector.tensor_mul(x2[:], frac_t[:], frac_t[:])
                        x3 = pool.tile([P, NC, 3], f32, tag="x3")
                        nc.vector.tensor_mul(x3[:], x2[:], frac_t[:])
                        o2 = pool.tile([P, NC, 3], f32, tag="o2")
                        nc.vector.tensor_mul(o2[:], omx[:], omx[:])
                        o3 = pool.tile([P, NC, 3], f32, tag="o3")
                        nc.vector.tensor_mul(o3[:], o2[:], omx[:])
                        SIX = 1.0 / 6.0
                        nc.vector.tensor_scalar_mul(W[:, :, :, 0], o3[:], SIX)
                        nc.vector.tensor_scalar_mul(W[:, :, :, 3], x3[:], SIX)
                        c1a = pool.tile([P, NC, 3], f32, tag="c1a")
                        nc.vector.scalar_tensor_tensor(
                            c1a[:], x3[:], 0.5, x2[:], op0=AL.mult, op1=AL.subtract
                        )
                        nc.vector.tensor_scalar_add(W[:, :, :, 1], c1a[:], 2.0 / 3.0)
                        c2a = pool.tile([P, NC, 3], f32, tag="c2a")
                        nc.vector.scalar_tensor_tensor(
                            c2a[:], o3[:], 0.5, o2[:], op0=AL.mult, op1=AL.subtract
                        )
                        nc.vector.tensor_scalar_add(W[:, :, :, 2], c2a[:], 2.0 / 3.0)
                        # bf16 z-weights for stage-1
                        Wzb = pool.tile([P, NC, 4], bf16, tag="Wzb")
                        nc.vector.tensor_copy(
                            Wzb[:],
                            sap(W[:], [[NC * 12, P], [12, NC], [1, 4]], 2 * 4),
                        )

                        # cell*64 (f32 exact), base = ti-1 per dim
                        bx = tif_t[:, :, 0]
                        by = tif_t[:, :, 1]
                        bz = tif_t[:, :, 2]
                        f1 = pool.tile([P, NC], f32, tag="f1")
                        nc.vector.scalar_tensor_tensor(
                            f1[:], by, 128.0, bz, op0=AL.mult, op1=AL.add
                        )
                        f2 = pool.tile([P, NC], f32, tag="f2")
                        nc.vector.scalar_tensor_tensor(
                            f2[:], bx, 16384.0, f1[:], op0=AL.mult, op1=AL.add
                        )
                        # cells use base=ti-1: subtract (1*16384+1*128+1), x64
                        basef = pool.tile([P, NC], f32, tag="basef")
                        nc.vector.tensor_scalar(
                            basef[:], f2[:], -16513.0, 64.0,
                            op0=AL.add, op1=AL.mult,
                        )
                        idxi = pool.tile([P, NC], i32, tag="idxi")
                        nc.vector.tensor_copy(idxi[:], basef[:])

                        # gather: one 64-value bf16 row per slot
                        X = xpool.tile([P, NC, 64], bf16, tag="X")
                        for n in range(NC):
                            nc.gpsimd.indirect_dma_start(
                                out=X[:, n, :],
                                out_offset=None,
                                in_=t64[:],
                                in_offset=bass.IndirectOffsetOnAxis(
                                    ap=idxi[:, n : n + 1], axis=0
                                ),
                            )

                        # contraction: z, y, x staged ((cx,cy) folded to one dim)
                        m1 = pool.tile([P, NC, 16, 4], bf16, tag="m1")
                        wzv = sap(
                            Wzb[:],
                            [[NC * 4, P], [4, NC], [0, 16], [1, 4]],
                        )
                        nc.vector.tensor_tensor(
                            m1[:],
                            sap(X[:], [[NC * 64, P], [64, NC], [4, 16], [1, 4]]),
                            wzv,
                            op=AL.mult,
                        )
                        A = pool.tile([P, NC, 4, 4], f32, tag="A")
                        nc.vector.tensor_reduce(
                            A[:].rearrange("p n i j -> p (n i j)"),
                            m1[:].rearrange("p n ij k -> p (n ij) k"),
                            axis=mybir.AxisListType.X,
                            op=AL.add,
                        )
                        wy = sap(W[:], [[NC * 12, P], [12, NC], [0, 4], [1, 4]], 1 * 4)
                        m2 = pool.tile([P, NC, 4, 4], f32, tag="m2")
                        nc.vector.tensor_tensor(m2[:], A[:], wy, op=AL.mult)
                        B = pool.tile([P, NC, 4], f32, tag="B")
                        nc.vector.tensor_reduce(
                            B[:].rearrange("p n i -> p (n i)"),
                            m2[:].rearrange("p n i j -> p (n i) j"),
                            axis=mybir.AxisListType.X,
                            op=AL.add,
                        )
                        wx = sap(W[:], [[NC * 12, P], [12, NC], [1, 4]])
                        m3 = pool.tile([P, NC, 4], f32, tag="m3")
                        nc.vector.tensor_tensor(m3[:], B[:], wx, op=AL.mult)
                        v = pool.tile([P, NC], f32, tag="v")
                        nc.vector.tensor_reduce(
                            v[:], m3[:], axis=mybir.AxisListType.X, op=AL.add
                        )
                        dst = sap(out_d[:], [[SLOTS, P], [1, NC]], c * NC)
                        nc.sync.dma_start(dst, v[:])

            if reps == 1:
                body()
            else:
                with tc.For_i(0, reps, 1):
                    body()

    nc.compile()
    return nc


def _prep_inputs(pts, control_pts):
    import ml_dtypes

    pts = np.ascontiguousarray(pts, dtype=np.float32)
    gbf = (
        np.ascontiguousarray(control_pts, np.float32)
        .reshape(GRID, GP)
        .astype(ml_dtypes.bfloat16)
    )
    in_maps = []
    for k in range(8):
        sl = pts[k * NPTS_CORE : (k + 1) * NPTS_CORE]
        pad = np.zeros((NPAD_CORE, 3), np.float32)
        pad[: sl.shape[0]] = sl
        in_maps.append({"pts": pad, "gbf": gbf})
    return in_maps


def kernel(pts: np.ndarray, control_pts: np.ndarray) -> np.ndarray:
    from concourse.bass_utils import run_bass_kernel_spmd

    if "nc" not in _CACHE:
        _CACHE["nc"] = build_program()
    nc = _CACHE["nc"]

    in_maps = _prep_inputs(pts, control_pts)
    res = run_bass_kernel_spmd(nc, in_maps, core_ids=list(range(8)))
    outs = []
    for k in range(8):
        o = res.results[k]["out"].reshape(NPAD_CORE)
        outs.append(o[:NPTS_CORE])
    return np.concatenate(outs).reshape(-1, 1)
